# revision 1
# baseline (speedup 1.0000x reference)
"""Causal dense attention (key=value) on 8 TRN2 NeuronCores.

Reference semantics (B=4, T=2048, D=1024, fp32):
    scores  = Q @ V^T                      [B, T, T]
    scores -= 1e9 * (~tril)                causal mask
    W       = softmax(scores, axis=-1)
    out     = W @ V                        [B, T, D]

Sharding: 2 cores per batch. Each batch's 16 causal q-tiles (128 rows
each, kv extent 128*(t+1)) are split odd/even so both cores get the
same padded kv-extent schedule EXT = [256, 512, ..., 2048] (ascending),
making the Bass program identical across all 8 cores (pure SPMD).
Padding columns are killed by the additive causal mask.

Host stages per core: Q^T (d-major) and V^T (d-major) pre-rounded to
f32r (full-rate TensorE), V (natural, bf16), and additive causal masks
for the last 256 columns of each slot.

Device pipeline per slot (software-pipelined, lag 2):
  mm1  S = Q^T.T @ V^T into PSUM kilo-windows (f32r), with the causal
       mask folded into the accumulation group as an identity-weighted
       matmul (S += I.T @ mask);
  stats row-max (DVE reduce, negated) -> exp with fused bias and
       accumulated row-sum (ScalarE) -> W (bf16) in SBUF;
  mm2  PE-transpose of W blocks, W^T.T @ V (bf16) into PSUM, then a
       ScalarE copy fused with the 1/rowsum scale, and DMA out.
Input DMAs are coalesced (one strided descriptor-set per wave) and
ordered by slot consumption so compute starts ~2 MiB in.
"""

import numpy as np

import concourse.bass as bass
import concourse.mybir as mybir
from concourse import bacc, tile
from concourse.bass_utils import run_bass_kernel_spmd
from concourse.masks import make_identity

import ml_dtypes

B, T, D = 4, 2048, 1024
NCORES = 8
NSLOT = 8
EXT = [256 * (j + 1) for j in range(NSLOT)]  # kv extent per slot
# processing order: smallest slot first (fast start), small slot last (short
# pipeline tail); big slots in the middle keep the PE pipeline saturated
ORDER = [1, 2, 3, 4, 5, 6, 7, 0]
NEG_INF = 1e9

F32 = mybir.dt.float32
F32R = mybir.dt.float32r
BF16 = mybir.dt.bfloat16

# matmul dtypes (tuning knobs)
MM1_DT = F32R   # scores matmul
MM2_BF16 = True  # weights/value matmul in bf16
REPEAT = 1      # >1: repeat the whole pipeline in-program (bench only)


def _round_f32r(x):
    """Round fp32 to 11 mantissa bits (RNE) — matches walrus fp32_to_fp32r."""
    u = np.asarray(x, dtype=np.float32).view(np.uint32)
    u = u + 0x7FF + ((u >> 12) & 1)
    u &= np.uint32(0xFFFFF000)
    return u.view(np.float32)


def _tiles_for_core(c):
    """q-tile index (within the batch) for each slot, for core c."""
    if c < 4:
        return [2 * j + 1 for j in range(NSLOT)]  # extents exactly EXT
    return [2 * j for j in range(NSLOT)]  # extents EXT - 128 (padded)


def _build_program():
    nc = bacc.Bacc("TRN2", target_bir_lowering=False)

    qt_d = nc.dram_tensor("qt", [D, NSLOT * 128], MM1_DT, kind="ExternalInput")
    vt_d = nc.dram_tensor("vt", [D, T], MM1_DT, kind="ExternalInput")
    v_d = nc.dram_tensor(
        "v", [T, D], BF16 if MM2_BF16 else F32, kind="ExternalInput"
    )
    mask_d = nc.dram_tensor("mask", [NSLOT, 128, 256], MM1_DT, kind="ExternalInput")
    identr_d = nc.dram_tensor("identr", [128, 128], MM1_DT, kind="ExternalInput")
    o_d = nc.dram_tensor("o", [NSLOT * 128, D], F32, kind="ExternalOutput")

    v_dt = BF16 if MM2_BF16 else F32

    with tile.TileContext(nc) as tc:
        with (
            tc.tile_pool(name="const", bufs=1) as constp,
            tc.tile_pool(name="qt", bufs=1) as qtp,
            tc.tile_pool(name="vt", bufs=1) as vtp,
            tc.tile_pool(name="vn", bufs=1) as vnp,
            tc.tile_pool(name="w", bufs=3) as wp,
            tc.tile_pool(name="wt", bufs=8) as wtp,
            tc.tile_pool(name="osb", bufs=2) as op,
            tc.tile_pool(name="stats", bufs=24) as statp,
            tc.tile_pool(name="ps_s", bufs=2, space="PSUM") as ps_s,
            tc.tile_pool(name="ps_t", bufs=2, space="PSUM") as ps_t,
            tc.tile_pool(name="ps_o", bufs=1, space="PSUM") as ps_o,
        ):
            ident = constp.tile([128, 128], BF16, tag="ident")
            make_identity(nc, ident[:])
            ident_r = constp.tile([128, 128], MM1_DT, tag="identr")

            # ACT exp-table warm-up: load exp_and_others during initial DMAs
            warm = statp.tile([128, 1], F32, tag="warm")
            nc.gpsimd.memset(warm[:], 0.0)
            nc.scalar.activation(warm[:], warm[:], mybir.ActivationFunctionType.Exp)

            masks = constp.tile([128, NSLOT * 256], MM1_DT, tag="masks")
            w_dt = BF16 if MM2_BF16 else F32

            # Resident inputs, DMA'd in slot-consumption order (per ORDER):
            # each slot first needs its Q^T quarter, V^T chunks and V rows up
            # to its extent; the masks ride along after the first slot's data.
            qts = {}   # (d8, j) -> [128, 128] AP
            vts = {}   # (d8, kc) -> [128, 512]
            vns = {}   # kb -> [128, D]

            def emit_dma_waves(j, first_rep):
                # one coalesced DMA per wave: SBUF [128, 8, w] <- DRAM
                # [(8*128), w] with d8 stacked along the free dim
                if (0, j) not in qts:
                    # early waves small (fast pipeline start), later 512-wide
                    if j < 2:
                        c0, cw = 0, 256
                    elif j < 4:
                        c0, cw = 256, 256
                    else:
                        c0, cw = 512, 512
                    t_ = qtp.tile([128, 8, cw], MM1_DT, tag=f"qtw{c0}")
                    nc.sync.dma_start(
                        t_[:], qt_d[:, c0 : c0 + cw].rearrange("(a p) q -> p a q", p=128)
                    )
                    for d8 in range(8):
                        for jj in range(c0 // 128, (c0 + cw) // 128):
                            qts[(d8, jj)] = t_[
                                :, d8,
                                (jj - c0 // 128) * 128 : (jj - c0 // 128 + 1) * 128,
                            ]
                for kc in range((EXT[j] + 511) // 512):
                    if (0, kc) in vts:
                        continue
                    t_ = vtp.tile([128, 8, 512], MM1_DT, tag=f"vtw{kc}")
                    if kc == 0:
                        # split the first chunk so the opening slot's
                        # matmuls start after ~1MiB instead of 2MiB
                        for hh in (0, 256):
                            nc.sync.dma_start(
                                t_[:, :, hh : hh + 256],
                                vt_d[:, hh : hh + 256].rearrange(
                                    "(a p) k -> p a k", p=128
                                ),
                            )
                    else:
                        nc.sync.dma_start(
                            t_[:],
                            vt_d[:, kc * 512 : (kc + 1) * 512].rearrange(
                                "(a p) k -> p a k", p=128
                            ),
                        )
                    for d8 in range(8):
                        vts[(d8, kc)] = t_[:, d8, :]
                if first_rep and j == ORDER[0]:
                    # needed only at the tail of the first mm1 group — keep
                    # these off the head of the DMA queue
                    nc.sync.dma_start(
                        masks[:].rearrange("p (j c) -> p j c", j=NSLOT),
                        mask_d.rearrange("j p c -> p j c"),
                    )
                    nc.sync.dma_start(ident_r[:], identr_d[:])

            def emit_front(j):
                """mm1 + softmax stats + exp for slot j. Returns (j, w_sb, rinv)."""
                E = EXT[j]
                # kilo-windows of up to 1024 columns (each a [128,1024] PSUM
                # tile = 2 banks; matmuls still issue per 512-column bank)
                kws = [(c0, min(1024, E - c0)) for c0 in range(0, E, 1024)]

                # sub-pass width: the opening slot streams 256-wide so its
                # first matmuls only wait on the first 1MiB V^T half-DMA
                sw_ = 256 if j == ORDER[0] else 512

                s_tiles = []
                for c0, kwd in kws:
                    s_ = ps_s.tile([128, 1024], F32, tag="sw")
                    for h0 in range(0, kwd, 512):
                        hw = min(512, kwd - h0)
                        a0 = c0 + h0  # absolute column
                        # the additive causal mask (last 256 columns of the
                        # slot) rides the accumulation group as an extra
                        # identity-weighted matmul: S[q,c] += I[k,q]*mask[k,c]
                        last_half = c0 + h0 + hw == E
                        for g0 in range(0, hw, sw_):
                            gw = min(sw_, hw - g0)
                            for d8 in range(8):
                                nc.tensor.matmul(
                                    s_[:, h0 + g0 : h0 + g0 + gw],
                                    qts[(d8, j)],
                                    vts[(d8, a0 // 512)][
                                        :,
                                        a0 % 512 + g0 : a0 % 512 + g0 + gw,
                                    ],
                                    start=(d8 == 0 and g0 == 0),
                                    stop=(
                                        d8 == 7
                                        and g0 + gw == hw
                                        and not last_half
                                    ),
                                )
                        if last_half:
                            nc.tensor.matmul(
                                s_[:, h0 + hw - 256 : h0 + hw],
                                ident_r[:],
                                masks[:, j * 256 : (j + 1) * 256],
                                start=False,
                                stop=True,
                            )
                    s_tiles.append(s_)

                # negated row max over the strip
                nmax = None
                for ki, (c0, kwd) in enumerate(kws):
                    nm = statp.tile([128, 1], F32, tag="nm")
                    nc.vector.reduce_max(
                        nm[:], s_tiles[ki][:, :kwd], axis=mybir.AxisListType.X,
                        negate=True,
                    )
                    if nmax is None:
                        nmax = nm
                    else:
                        nm2 = statp.tile([128, 1], F32, tag="nmc")
                        nc.vector.tensor_tensor(
                            nm2[:], nmax[:], nm[:], op=mybir.AluOpType.min
                        )
                        nmax = nm2

                # exp (fused bias & row-sum) -> W in SBUF
                w_sb = wp.tile([128, E], w_dt, tag="w")
                rsum = None
                for ki, (c0, kwd) in enumerate(kws):
                    rs = statp.tile([128, 1], F32, tag="rs")
                    nc.scalar.activation(
                        w_sb[:, c0 : c0 + kwd],
                        s_tiles[ki][:, :kwd],
                        mybir.ActivationFunctionType.Exp,
                        bias=nmax[:],
                        accum_out=rs[:],
                    )
                    if rsum is None:
                        rsum = rs
                    else:
                        rs2 = statp.tile([128, 1], F32, tag="rsc")
                        nc.vector.tensor_add(rs2[:], rsum[:], rs[:])
                        rsum = rs2
                rinv = statp.tile([128, 1], F32, tag="rinv")
                nc.vector.reciprocal(rinv[:], rsum[:])
                return (j, w_sb, rinv)

            def emit_vn_waves(j):
                for q_ in range((EXT[j] // 128 + 3) // 4):
                    if q_ * 4 in vns:
                        continue
                    t_ = vnp.tile([128, 4, D], v_dt, tag=f"vnw{q_}")
                    nc.sync.dma_start(
                        t_[:],
                        v_d[q_ * 512 : (q_ + 1) * 512, :].rearrange(
                            "(a p) d -> p a d", p=128
                        ),
                    )
                    for kb in range(q_ * 4, q_ * 4 + 4):
                        vns[kb] = t_[:, kb % 4, :]

            def emit_back(state):
                """transpose W, mm2, normalize, store for a finished slot."""
                j, w_sb, rinv = state
                E = EXT[j]
                nblk = E // 128
                wt_tiles = []
                for g0 in range(0, nblk, 4):
                    gn = min(4, nblk - g0)
                    t_ps = ps_t.tile([128, 512], w_dt, tag="tp")
                    for bi in range(gn):
                        blk = g0 + bi
                        nc.tensor.transpose(
                            t_ps[:, bi * 128 : (bi + 1) * 128],
                            w_sb[:, blk * 128 : (blk + 1) * 128],
                            ident[:],
                        )
                    wt_sb = wtp.tile([128, 512], w_dt, tag="wt")
                    nc.vector.tensor_copy(wt_sb[:, : gn * 128], t_ps[:, : gn * 128])
                    wt_tiles.append(wt_sb)

                # output matmul: O[q, d] = W^T.T @ V
                o_ps = ps_o.tile([128, D], F32, tag="op")
                for blk in range(nblk):
                    wt_ap = wt_tiles[blk // 4][
                        :, (blk % 4) * 128 : (blk % 4 + 1) * 128
                    ]
                    v_ap = vns[blk]
                    if not MM2_BF16:
                        wt_ap = wt_ap.bitcast(F32R)
                    for dd in range(0, D, 512):
                        va = v_ap[:, dd : dd + 512]
                        if not MM2_BF16:
                            va = va.bitcast(F32R)
                        nc.tensor.matmul(
                            o_ps[:, dd : dd + 512],
                            wt_ap,
                            va,
                            start=(blk == 0),
                            stop=(blk == nblk - 1),
                        )

                o_sb = op.tile([128, D], F32, tag="o")
                for dd in range(0, D, 512):
                    nc.scalar.activation(
                        o_sb[:, dd : dd + 512],
                        o_ps[:, dd : dd + 512],
                        mybir.ActivationFunctionType.Copy,
                        scale=rinv[:],
                    )
                    nc.sync.dma_start(
                        o_d[j * 128 : (j + 1) * 128, dd : dd + 512],
                        o_sb[:, dd : dd + 512],
                    )

            # software pipeline (lag 2): slot j's scores/softmax overlap the
            # two previous slots' transpose+mm2 work queued on the PE
            pending = []
            for rep in range(REPEAT):
                if rep > 0:
                    # drain the pipeline before re-loading inputs (bench path)
                    for st in pending:
                        emit_back(st)
                    pending = []
                    qts.clear()
                    vts.clear()
                    vns.clear()
                for j in ORDER:
                    emit_dma_waves(j, rep == 0)
                    pending.append(emit_front(j))
                    emit_vn_waves(j)
                    if len(pending) > 2:
                        emit_back(pending.pop(0))
            for st in pending:
                emit_back(st)

    nc.finalize()
    return nc


_NC_CACHE = None


def _get_program():
    global _NC_CACHE
    if _NC_CACHE is None:
        _NC_CACHE = _build_program()
    return _NC_CACHE


def stage_inputs(query, value):
    """Build the 8 per-core input maps from the full query/value arrays."""
    query = np.asarray(query, dtype=np.float32)
    value = np.asarray(value, dtype=np.float32)

    in_maps = []
    for c in range(NCORES):
        b = c % 4
        tiles = _tiles_for_core(c)

        q_rows = np.concatenate(
            [query[b, t * 128 : (t + 1) * 128, :] for t in tiles], axis=0
        )  # [1024, D]
        qt = np.ascontiguousarray(q_rows.T)  # [D, 1024]
        vt = np.ascontiguousarray(value[b].T)  # [D, T]
        if MM1_DT == F32R:
            qt = _round_f32r(qt)
            vt = _round_f32r(vt)
        vn = value[b]
        if MM2_BF16:
            vn = vn.astype(ml_dtypes.bfloat16)
        vn = np.ascontiguousarray(vn)

        mask = np.zeros((NSLOT, 128, 256), dtype=np.float32)
        for j in range(NSLOT):
            t = tiles[j]
            rows = t * 128 + np.arange(128)[:, None]  # global q row
            cols = EXT[j] - 256 + np.arange(256)[None, :]  # global kv col
            mask[j][cols > rows] = -NEG_INF

        identr = np.eye(128, dtype=np.float32)

        in_maps.append(
            {"qt": qt, "vt": vt, "v": vn, "mask": mask, "identr": identr}
        )
    return in_maps


def kernel(query, value):
    nc = _get_program()
    in_maps = stage_inputs(query, value)
    res = run_bass_kernel_spmd(nc, in_maps, core_ids=list(range(NCORES)))

    out = np.empty((B, T, D), dtype=np.float32)
    for c in range(NCORES):
        o = res.results[c]["o"]  # [1024, D]
        b = c % 4
        for j, t in enumerate(_tiles_for_core(c)):
            out[b, t * 128 : (t + 1) * 128, :] = o[j * 128 : (j + 1) * 128, :]
    return out



# revision 50
# speedup vs baseline: 1.3132x; 1.3132x over previous
"""Causal dense attention (key=value) on 8 TRN2 NeuronCores.

Reference semantics (B=4, T=2048, D=1024, fp32):
    scores  = Q @ V^T                      [B, T, T]
    scores -= 1e9 * (~tril)                causal mask
    W       = softmax(scores, axis=-1)
    out     = W @ V                        [B, T, D]

Sharding: 2 cores per batch; each batch's 16 causal q-tiles (128 rows)
split odd/even across the pair so all 8 cores run one SPMD program with
the padded kv-extent schedule EXT = [256, 512, ..., 2048].

v2 (cost-model driven):
  - fp16 operands everywhere (mm1, mm2, transposes, output); host converts.
  - inputs staged as partition-major SBUF images so every DMA is fully
    contiguous (128 descriptors, >=2KB each).
  - all input DMAs issued up-front in slot-consumption order; output DMAs
    queue behind them on the same ring.
  - PE warm-up transposes cover the initial DMA latency so real matmuls
    start at full clock.
  - per-slot pipeline (lag 3): transposes of a finished slot are emitted
    BEFORE the next front's matmuls, mm2 after, so the W^T copies hide
    under mm1 instead of stalling the PE.
  - causal mask folded into the mm1 accumulation group as an
    identity-weighted matmul (additive -30000 mask, fp16-safe).
"""

import numpy as np

import concourse.bass as bass
import concourse.mybir as mybir
from concourse import bacc, tile
from concourse.bass_utils import run_bass_kernel_spmd
from concourse.masks import make_identity

B, T, D = 4, 2048, 1024
NCORES = 8
NSLOT = 8
EXT = [256 * (j + 1) for j in range(NSLOT)]  # kv extent per slot
# ascending: tiny slots first (fast, low-bandwidth ramp-in), biggest last
ORDER = [0, 1, 2, 3, 4, 5, 6, 7]
LAG = 3
NWARM = 44
MASK_VAL = -30000.0

F32 = mybir.dt.float32
F16 = mybir.dt.float16

# Input DMA waves: each is ONE contiguous partition-major DMA packing
# several logical chunks. Few, larger waves keep the SP sequencer's DMA
# issue phase short so the W^T transpose-DMAs never interleave ahead of
# inputs on the shared HWDGE completion rings.
#   chunk kinds: ("mask",), ("qt", j), ("vt", c0, w), ("vn", r0, nrow)
WAVES = [
    [("mask",), ("qt", 0), ("vt", 0, 128), ("vt", 128, 128)],
    [("qt", 1), ("vt", 256, 256)],
    [("qt", 2), ("vt", 512, 256)],
    [("qt", 3), ("vt", 768, 256)],
    [("qt", 4), ("vt", 1024, 256), ("vn", 0, 2)],
    [("qt", 5), ("vt", 1280, 256), ("vn", 256, 2)],
    [("vn", 512, 4)],
    [("qt", 6), ("vt", 1536, 256)],
    [("qt", 7), ("vt", 1792, 256)],
    [("vn", 1024, 4)],
    [("vn", 1536, 4)],
]


def _chunk_cols(ch):
    if ch[0] == "mask":
        return 256
    if ch[0] == "qt":
        return 8 * 128
    if ch[0] == "vt":
        return 8 * ch[2]
    return ch[2] * D  # vn


def _wave_cols(wave):
    return sum(_chunk_cols(ch) for ch in wave)


def _tiles_for_core(c):
    """q-tile index (within the batch) for each slot, for core c."""
    if c < 4:
        return [2 * j + 1 for j in range(NSLOT)]  # extents exactly EXT
    return [2 * j for j in range(NSLOT)]  # extents EXT - 128 (padded)


VT_SPANS = [(ch[1], ch[2]) for wave in WAVES for ch in wave if ch[0] == "vt"]


def _segments(c0, w):
    """Split window [c0, c0+w) at vt-chunk boundaries -> (span, off, g0, gw)."""
    segs = []
    for si, (s0, sw) in enumerate(VT_SPANS):
        lo = max(c0, s0)
        hi = min(c0 + w, s0 + sw)
        if lo < hi:
            segs.append((si, lo - s0, lo - c0, hi - lo))
    return segs


def _build_program():
    nc = bacc.Bacc("TRN2", target_bir_lowering=False)

    wave_d = [nc.dram_tensor(f"w{k}", [128, _wave_cols(wave)], F16,
                             kind="ExternalInput")
              for k, wave in enumerate(WAVES)]
    o_d = nc.dram_tensor("o", [NSLOT * 128, D], F16, kind="ExternalOutput")

    with tile.TileContext(nc) as tc:
        with (
            tc.tile_pool(name="const", bufs=1) as constp,
            tc.tile_pool(name="vt", bufs=1) as vtp,
            tc.tile_pool(name="w", bufs=4) as wp,
            tc.tile_pool(name="wt", bufs=8) as wtp,
            tc.tile_pool(name="osb", bufs=8) as op,
            tc.tile_pool(name="stats", bufs=24) as statp,
            tc.tile_pool(name="ps_s", bufs=4, space="PSUM") as ps_s,
            tc.tile_pool(name="ps_t", bufs=2, space="PSUM") as ps_t,
            tc.tile_pool(name="ps_o", bufs=2, space="PSUM") as ps_o,
        ):
            # warm-source: zeros are fine, warm-up results are never read
            warmsrc = constp.tile([128, 128], F16, tag="warmsrc")
            nc.gpsimd.memset(warmsrc[:], 0.0)

            ident = constp.tile([128, 128], F16, tag="ident")
            make_identity(nc, ident[:])

            # ACT exp-table warm-up during initial DMAs
            warm_a = statp.tile([128, 1], F32, tag="warma")
            nc.gpsimd.memset(warm_a[:], 0.0)
            nc.scalar.activation(warm_a[:], warm_a[:],
                                 mybir.ActivationFunctionType.Exp)

            # ---- all input DMAs, in slot-consumption order -------------
            qts = {}   # j -> AP [128, 8*128]   (d8-major q-tile image)
            vtc = []   # vt span idx -> (tile, off, w)
            vnb = {}   # 128-row block index -> AP [128, D]
            maskc = None

            for k, wave in enumerate(WAVES):
                t_ = vtp.tile([128, _wave_cols(wave)], F16, tag=f"wv{k}")
                nc.sync.dma_start(t_[:], wave_d[k][:])
                off = 0
                for ch in wave:
                    cols = _chunk_cols(ch)
                    if ch[0] == "mask":
                        maskc = t_[:, off:off + 256]
                    elif ch[0] == "qt":
                        qts[ch[1]] = (t_, off)
                    elif ch[0] == "vt":
                        vtc.append((t_, off, ch[2]))
                    else:  # vn
                        for i in range(ch[2]):
                            vnb[ch[1] // 128 + i] = t_[:, off + i * D:
                                                       off + (i + 1) * D]
                    off += cols

            # ---- PE warm-up: junk transposes cover the DMA head --------
            for _ in range(NWARM):
                t_ps = ps_t.tile([128, 512], F16, tag="tp",
                                 padded_shape=[128, 1024])
                nc.tensor.transpose(t_ps[:, 0:128], warmsrc[:], warmsrc[:])

            def emit_front(j):
                """mm1 + softmax stats + exp for slot j."""
                E = EXT[j]
                windows = [(c0, min(512, E - c0)) for c0 in range(0, E, 512)]
                nmax = None
                s_list = []
                for c0, w in windows:
                    s_ = ps_s.tile([128, 512], F32, tag="sw")
                    last = (c0 + w == E)
                    segs = _segments(c0, w)
                    qt_t, qt_off = qts[j]
                    for si, (sp, off, g0, gw) in enumerate(segs):
                        vt_t, vt_off, vw = vtc[sp]
                        for d8 in range(8):
                            nc.tensor.matmul(
                                s_[:, g0:g0 + gw],
                                qt_t[:, qt_off + d8 * 128:
                                     qt_off + (d8 + 1) * 128],
                                vt_t[:, vt_off + d8 * vw + off:
                                     vt_off + d8 * vw + off + gw],
                                start=(d8 == 0 and g0 == 0),
                                stop=(si == len(segs) - 1 and d8 == 7),
                            )
                    # additive causal mask on the last 256 columns (DVE)
                    if last:
                        nc.vector.tensor_add(s_[:, w - 256:w],
                                             s_[:, w - 256:w], maskc[:])
                    nm = statp.tile([128, 1], F32, tag="nm")
                    nc.vector.reduce_max(nm[:], s_[:, :w],
                                         axis=mybir.AxisListType.X, negate=True)
                    if nmax is None:
                        nmax = nm
                    else:
                        nm2 = statp.tile([128, 1], F32, tag="nmc")
                        nc.vector.tensor_tensor(nm2[:], nmax[:], nm[:],
                                                op=mybir.AluOpType.min)
                        nmax = nm2
                    s_list.append((c0, w, s_))

                w_sb = wp.tile([128, E], F16, tag="w")
                rsum = None
                for c0, w, s_ in s_list:
                    rs = statp.tile([128, 1], F32, tag="rs")
                    nc.scalar.activation(
                        w_sb[:, c0:c0 + w], s_[:, :w],
                        mybir.ActivationFunctionType.Exp,
                        bias=nmax[:], accum_out=rs[:],
                    )
                    if rsum is None:
                        rsum = rs
                    else:
                        rs2 = statp.tile([128, 1], F32, tag="rsc")
                        nc.vector.tensor_add(rs2[:], rsum[:], rs[:])
                        rsum = rs2
                return [j, w_sb, rsum, None, None]

            def emit_back_T(state):
                """PE-transpose W blocks (groups of 4) + DVE copy to SBUF.
                Emitted one iteration before the slot's mm2, so the DVE
                copies complete under the next front's mm1."""
                j, w_sb, rsum, _, _ = state
                # 1/rowsum on DVE — but deferred to here: rsum is long
                # since final, so this never head-of-line-blocks the DVE
                # stats stream the way it would inside emit_front
                rinv = statp.tile([128, 1], F32, tag="rinv")
                nc.vector.reciprocal(rinv[:], rsum[:])
                state[3] = rinv
                nblk = EXT[j] // 128
                wts = []
                for g0 in range(0, nblk, 4):
                    gn = min(4, nblk - g0)
                    t_ps = ps_t.tile([128, 512], F16, tag="tp",
                                     padded_shape=[128, 1024])
                    for bi in range(gn):
                        blk = g0 + bi
                        nc.tensor.transpose(
                            t_ps[:, bi * 128:(bi + 1) * 128],
                            w_sb[:, blk * 128:(blk + 1) * 128],
                            ident[:],
                        )
                    wt = wtp.tile([128, 512], F16, tag="wt")
                    nc.vector.tensor_copy(wt[:, :gn * 128], t_ps[:, :gn * 128])
                    wts.append(wt)
                state[4] = wts

            def emit_back_mm2(state, fine_tail=False):
                """mm2 (half-D passes), normalize, store.

                fine_tail: narrow final passes — used for the last drained
                slot so the post-PE evac+DMA+sem chain is as short as
                possible.
                """
                j, w_sb, rsum, rinv, wts = state
                nblk = EXT[j] // 128
                passes = ((0, 512), (512, 256), (768, 256)) \
                    if fine_tail else ((0, 512), (512, 512))
                for pi, (dd, dw) in enumerate(passes):
                    o_ps = ps_o.tile([128, 512], F32, tag="op")
                    for blk in range(nblk):
                        nc.tensor.matmul(
                            o_ps[:, :dw],
                            wts[blk // 4][:, (blk % 4) * 128:
                                          (blk % 4 + 1) * 128],
                            vnb[blk][:, dd:dd + dw],
                            start=(blk == 0),
                            stop=(blk == nblk - 1),
                        )
                    o_sb = op.tile([128, 512], F16, tag="o")
                    nc.scalar.activation(
                        o_sb[:, :dw], o_ps[:, :dw],
                        mybir.ActivationFunctionType.Copy, scale=rinv[:],
                    )
                    nc.sync.dma_start(
                        o_d[j * 128:(j + 1) * 128, dd:dd + dw], o_sb[:, :dw])

            # ---- main pipeline -----------------------------------------
            # T (W^T transpose-DMA) at i-2, mm2 at i-3: the T-DMA is
            # emitted before the previous slot's output DMAs so their
            # data waits never hold it up on the SP sequencer.
            states = []
            for i, j in enumerate(ORDER):
                states.append(emit_front(j))
                if i >= 2:
                    emit_back_T(states[i - 2])
                if i >= LAG:
                    emit_back_mm2(states[i - LAG])
            # drain. ORDER is ascending so the largest slot goes last:
            # its long mm2 hides every earlier slot's output DMA, and
            # only its own (narrowed) final pass sits in the post-PE tail.
            n = len(states)
            emit_back_T(states[n - 2])
            emit_back_T(states[n - 1])
            for k in range(n - LAG, n):
                emit_back_mm2(states[k], fine_tail=(k == n - 1))

    nc.finalize()
    return nc


_NC_CACHE = None


def _get_program():
    global _NC_CACHE
    if _NC_CACHE is None:
        _NC_CACHE = _build_program()
    return _NC_CACHE


def stage_inputs(query, value):
    """Build the 8 per-core input maps (partition-major fp16 images)."""
    query = np.asarray(query, dtype=np.float32)
    value = np.asarray(value, dtype=np.float32)

    in_maps = []
    for c in range(NCORES):
        b = c % 4
        tiles = _tiles_for_core(c)
        Q = query[b]
        V = value[b]
        # mask threshold is slot-independent: c - r > t*128 + 256 - EXT
        thr = 128 if c < 4 else 0
        r = np.arange(128)[:, None]
        cc = np.arange(256)[None, :]
        mask_img = np.where(cc - r > thr, MASK_VAL, 0.0).astype(np.float16)

        def chunk_img(ch):
            if ch[0] == "mask":
                return mask_img
            if ch[0] == "qt":
                t = tiles[ch[1]]
                qtile = Q[t * 128:(t + 1) * 128, :]  # [128q, 1024d]
                # per-partition layout: [a(d-chunk) major, q-col minor]
                return qtile.T.reshape(8, 128, 128).transpose(1, 0, 2) \
                    .reshape(128, 8 * 128)
            if ch[0] == "vt":
                c0, w = ch[1], ch[2]
                return V[c0:c0 + w, :].T.reshape(8, 128, w) \
                    .transpose(1, 0, 2).reshape(128, 8 * w)
            r0, n = ch[1], ch[2]
            return V[r0:r0 + n * 128, :].reshape(n, 128, D) \
                .transpose(1, 0, 2).reshape(128, n * D)

        m = {}
        for k, wave in enumerate(WAVES):
            m[f"w{k}"] = np.ascontiguousarray(np.hstack(
                [chunk_img(ch) for ch in wave])).astype(np.float16)
        in_maps.append(m)
    return in_maps


def kernel(query, value):
    nc = _get_program()
    in_maps = stage_inputs(query, value)
    res = run_bass_kernel_spmd(nc, in_maps, core_ids=list(range(NCORES)))

    out = np.empty((B, T, D), dtype=np.float32)
    for c in range(NCORES):
        o = np.asarray(res.results[c]["o"], dtype=np.float32)  # [1024, D]
        b = c % 4
        for j, t in enumerate(_tiles_for_core(c)):
            out[b, t * 128:(t + 1) * 128, :] = o[j * 128:(j + 1) * 128, :]
    return out


# revision 60
# speedup vs baseline: 1.3407x; 1.0209x over previous
"""Causal dense attention (key=value) on 8 TRN2 NeuronCores.

Reference semantics (B=4, T=2048, D=1024, fp32):
    scores  = Q @ V^T                      [B, T, T]
    scores -= 1e9 * (~tril)                causal mask
    W       = softmax(scores, axis=-1)
    out     = W @ V                        [B, T, D]

Sharding: 2 cores per batch; each batch's 16 causal q-tiles (128 rows)
split odd/even across the pair so all 8 cores run one SPMD program with
the padded kv-extent schedule EXT = [256, 512, ..., 2048].

v2 (cost-model driven):
  - fp16 operands everywhere (mm1, mm2, transposes, output); host converts.
  - inputs staged as partition-major SBUF images so every DMA is fully
    contiguous (128 descriptors, >=2KB each).
  - all input DMAs issued up-front in slot-consumption order; output DMAs
    queue behind them on the same ring.
  - PE warm-up transposes cover the initial DMA latency so real matmuls
    start at full clock.
  - per-slot pipeline (lag 3): transposes of a finished slot are emitted
    BEFORE the next front's matmuls, mm2 after, so the W^T copies hide
    under mm1 instead of stalling the PE.
  - causal mask folded into the mm1 accumulation group as an
    identity-weighted matmul (additive -30000 mask, fp16-safe).
"""

import numpy as np

import concourse.bass as bass
import concourse.mybir as mybir
from concourse import bacc, tile
from concourse.bass_utils import run_bass_kernel_spmd
from concourse.masks import make_identity

B, T, D = 4, 2048, 1024
NCORES = 8
NSLOT = 8
EXT = [256 * (j + 1) for j in range(NSLOT)]  # kv extent per slot
# ascending: tiny slots first (fast, low-bandwidth ramp-in), biggest last
ORDER = [0, 1, 2, 3, 4, 5, 6, 7]
LAG = 3
NWARM = 34
MASK_VAL = -30000.0

F32 = mybir.dt.float32
F16 = mybir.dt.float16

# Input DMA waves: each is ONE contiguous partition-major DMA packing
# several logical chunks. Few, larger waves keep the SP sequencer's DMA
# issue phase short so the W^T transpose-DMAs never interleave ahead of
# inputs on the shared HWDGE completion rings.
#   chunk kinds: ("mask",), ("qt", j), ("vt", c0, w), ("vn", r0, nrow)
WAVES = [
    [("qt", 0), ("vt", 0, 128), ("vt", 128, 128)],
    [("mask",), ("qt", 1)],
    [("vt", 256, 128)],
    [("vt", 384, 128)],
    [("qt", 2)],
    [("vt", 512, 256)],
    [("qt", 3)],
    [("vt", 768, 256)],
    [("qt", 4)],
    [("vt", 1024, 256), ("vn", 0, 2)],
    [("qt", 5), ("vt", 1280, 256), ("vn", 256, 2)],
    [("vn", 512, 4)],
    [("qt", 6), ("vt", 1536, 256)],
    [("qt", 7), ("vt", 1792, 256)],
    [("vn", 1024, 4)],
    [("vn", 1536, 4)],
]


def _chunk_cols(ch):
    if ch[0] == "mask":
        return 256
    if ch[0] == "qt":
        return 8 * 128
    if ch[0] == "vt":
        return 8 * ch[2]
    return ch[2] * D  # vn


def _wave_cols(wave):
    return sum(_chunk_cols(ch) for ch in wave)


def _tiles_for_core(c):
    """q-tile index (within the batch) for each slot, for core c."""
    if c < 4:
        return [2 * j + 1 for j in range(NSLOT)]  # extents exactly EXT
    return [2 * j for j in range(NSLOT)]  # extents EXT - 128 (padded)


VT_SPANS = [(ch[1], ch[2]) for wave in WAVES for ch in wave if ch[0] == "vt"]


def _segments(c0, w):
    """Split window [c0, c0+w) at vt-chunk boundaries -> (span, off, g0, gw)."""
    segs = []
    for si, (s0, sw) in enumerate(VT_SPANS):
        lo = max(c0, s0)
        hi = min(c0 + w, s0 + sw)
        if lo < hi:
            segs.append((si, lo - s0, lo - c0, hi - lo))
    return segs


def _build_program():
    nc = bacc.Bacc("TRN2", target_bir_lowering=False)

    wave_d = [nc.dram_tensor(f"w{k}", [128, _wave_cols(wave)], F16,
                             kind="ExternalInput")
              for k, wave in enumerate(WAVES)]
    o_d = nc.dram_tensor("o", [NSLOT * 128, D], F16, kind="ExternalOutput")

    with tile.TileContext(nc) as tc:
        with (
            tc.tile_pool(name="const", bufs=1) as constp,
            tc.tile_pool(name="vt", bufs=1) as vtp,
            tc.tile_pool(name="w", bufs=4) as wp,
            tc.tile_pool(name="wt", bufs=8) as wtp,
            tc.tile_pool(name="osb", bufs=8) as op,
            tc.tile_pool(name="stats", bufs=24) as statp,
            tc.tile_pool(name="ps_s", bufs=4, space="PSUM") as ps_s,
            tc.tile_pool(name="ps_t", bufs=2, space="PSUM") as ps_t,
            tc.tile_pool(name="ps_o", bufs=2, space="PSUM") as ps_o,
        ):
            # warm-source: zeros are fine, warm-up results are never read
            warmsrc = constp.tile([128, 128], F16, tag="warmsrc")
            nc.gpsimd.memset(warmsrc[:], 0.0)

            ident = constp.tile([128, 128], F16, tag="ident")
            make_identity(nc, ident[:])

            # ACT exp-table warm-up during initial DMAs
            warm_a = statp.tile([128, 1], F32, tag="warma")
            nc.gpsimd.memset(warm_a[:], 0.0)
            nc.scalar.activation(warm_a[:], warm_a[:],
                                 mybir.ActivationFunctionType.Exp)

            # ---- all input DMAs, in slot-consumption order -------------
            qts = {}   # j -> AP [128, 8*128]   (d8-major q-tile image)
            vtc = []   # vt span idx -> (tile, off, w)
            vnb = {}   # 128-row block index -> AP [128, D]
            maskc = None

            for k, wave in enumerate(WAVES):
                t_ = vtp.tile([128, _wave_cols(wave)], F16, tag=f"wv{k}")
                nc.sync.dma_start(t_[:], wave_d[k][:])
                off = 0
                for ch in wave:
                    cols = _chunk_cols(ch)
                    if ch[0] == "mask":
                        maskc = t_[:, off:off + 256]
                    elif ch[0] == "qt":
                        qts[ch[1]] = (t_, off)
                    elif ch[0] == "vt":
                        vtc.append((t_, off, ch[2]))
                    else:  # vn
                        for i in range(ch[2]):
                            vnb[ch[1] // 128 + i] = t_[:, off + i * D:
                                                       off + (i + 1) * D]
                    off += cols

            # ---- PE warm-up: junk transposes cover the DMA head --------
            for _ in range(NWARM):
                t_ps = ps_t.tile([128, 512], F16, tag="tp",
                                 padded_shape=[128, 1024])
                nc.tensor.transpose(t_ps[:, 0:128], warmsrc[:], warmsrc[:])

            def emit_junk(n):
                """junk transposes: keep the PE queue fed across known
                data-bound head gaps (53ns each, results never read)."""
                for _ in range(n):
                    t_ps = ps_t.tile([128, 512], F16, tag="tp",
                                     padded_shape=[128, 1024])
                    nc.tensor.transpose(t_ps[:, 0:128], warmsrc[:],
                                        warmsrc[:])

            def emit_front(j, junk_mid=0):
                """mm1 + softmax stats + exp for slot j."""
                E = EXT[j]
                windows = [(c0, min(512, E - c0)) for c0 in range(0, E, 512)]
                nmax = None
                s_list = []
                for c0, w in windows:
                    s_ = ps_s.tile([128, 512], F32, tag="sw")
                    last = (c0 + w == E)
                    segs = _segments(c0, w)
                    qt_t, qt_off = qts[j]
                    for si, (sp, off, g0, gw) in enumerate(segs):
                        vt_t, vt_off, vw = vtc[sp]
                        for d8 in range(8):
                            nc.tensor.matmul(
                                s_[:, g0:g0 + gw],
                                qt_t[:, qt_off + d8 * 128:
                                     qt_off + (d8 + 1) * 128],
                                vt_t[:, vt_off + d8 * vw + off:
                                     vt_off + d8 * vw + off + gw],
                                start=(d8 == 0 and g0 == 0),
                                stop=(si == len(segs) - 1 and d8 == 7),
                            )
                    # additive causal mask on the last 256 columns (DVE)
                    if last:
                        nc.vector.tensor_add(s_[:, w - 256:w],
                                             s_[:, w - 256:w], maskc[:])
                    nm = statp.tile([128, 1], F32, tag="nm")
                    nc.vector.reduce_max(nm[:], s_[:, :w],
                                         axis=mybir.AxisListType.X, negate=True)
                    if nmax is None:
                        nmax = nm
                    else:
                        nm2 = statp.tile([128, 1], F32, tag="nmc")
                        nc.vector.tensor_tensor(nm2[:], nmax[:], nm[:],
                                                op=mybir.AluOpType.min)
                        nmax = nm2
                    s_list.append((c0, w, s_))

                w_sb = wp.tile([128, E], F16, tag="w")
                rsum = None
                for c0, w, s_ in s_list:
                    rs = statp.tile([128, 1], F32, tag="rs")
                    nc.scalar.activation(
                        w_sb[:, c0:c0 + w], s_[:, :w],
                        mybir.ActivationFunctionType.Exp,
                        bias=nmax[:], accum_out=rs[:],
                    )
                    if rsum is None:
                        rsum = rs
                    else:
                        rs2 = statp.tile([128, 1], F32, tag="rsc")
                        nc.vector.tensor_add(rs2[:], rsum[:], rs[:])
                        rsum = rs2
                return [j, w_sb, rsum, None, None]

            def emit_back_T(state):
                """PE-transpose W blocks (groups of 4) + DVE copy to SBUF.
                Emitted one iteration before the slot's mm2, so the DVE
                copies complete under the next front's mm1."""
                j, w_sb, rsum, _, _ = state
                # 1/rowsum on DVE — but deferred to here: rsum is long
                # since final, so this never head-of-line-blocks the DVE
                # stats stream the way it would inside emit_front
                rinv = statp.tile([128, 1], F32, tag="rinv")
                nc.vector.reciprocal(rinv[:], rsum[:])
                state[3] = rinv
                nblk = EXT[j] // 128
                wts = []
                for g0 in range(0, nblk, 4):
                    gn = min(4, nblk - g0)
                    t_ps = ps_t.tile([128, 512], F16, tag="tp",
                                     padded_shape=[128, 1024])
                    for bi in range(gn):
                        blk = g0 + bi
                        nc.tensor.transpose(
                            t_ps[:, bi * 128:(bi + 1) * 128],
                            w_sb[:, blk * 128:(blk + 1) * 128],
                            ident[:],
                        )
                    wt = wtp.tile([128, 512], F16, tag="wt")
                    nc.vector.tensor_copy(wt[:, :gn * 128], t_ps[:, :gn * 128])
                    wts.append(wt)
                state[4] = wts

            def emit_back_mm2(state, fine_tail=False):
                """mm2 (half-D passes), normalize, store.

                fine_tail: narrow final passes — used for the last drained
                slot so the post-PE evac+DMA+sem chain is as short as
                possible.
                """
                j, w_sb, rsum, rinv, wts = state
                nblk = EXT[j] // 128
                passes = ((0, 512), (512, 384), (896, 128)) \
                    if fine_tail else ((0, 512), (512, 512))
                for pi, (dd, dw) in enumerate(passes):
                    o_ps = ps_o.tile([128, 512], F32, tag="op")
                    for blk in range(nblk):
                        nc.tensor.matmul(
                            o_ps[:, :dw],
                            wts[blk // 4][:, (blk % 4) * 128:
                                          (blk % 4 + 1) * 128],
                            vnb[blk][:, dd:dd + dw],
                            start=(blk == 0),
                            stop=(blk == nblk - 1),
                        )
                    o_sb = op.tile([128, 512], F16, tag="o")
                    nc.scalar.activation(
                        o_sb[:, :dw], o_ps[:, :dw],
                        mybir.ActivationFunctionType.Copy, scale=rinv[:],
                    )
                    nc.sync.dma_start(
                        o_d[j * 128:(j + 1) * 128, dd:dd + dw], o_sb[:, :dw])

            # ---- main pipeline -----------------------------------------
            # T (W^T transpose-DMA) at i-2, mm2 at i-3: the T-DMA is
            # emitted before the previous slot's output DMAs so their
            # data waits never hold it up on the SP sequencer.
            states = []
            for i, j in enumerate(ORDER):
                states.append(emit_front(j))
                if i >= 2:
                    emit_back_T(states[i - 2])
                if i >= LAG:
                    emit_back_mm2(states[i - LAG])
            # drain. ORDER is ascending so the largest slot goes last:
            # its long mm2 hides every earlier slot's output DMA, and
            # only its own (narrowed) final pass sits in the post-PE tail.
            n = len(states)
            for k in range(n - LAG, n):
                # T for the two slots the main loop didn't reach (n-2, n-1),
                # interleaved so each T's PSUM/copy chain hides under the
                # preceding slot's mm2
                if n - 2 <= k + 1 < n:
                    emit_back_T(states[k + 1])
                emit_back_mm2(states[k], fine_tail=(k == n - 1))

    nc.finalize()
    return nc


_NC_CACHE = None


def _get_program():
    global _NC_CACHE
    if _NC_CACHE is None:
        _NC_CACHE = _build_program()
    return _NC_CACHE


def stage_inputs(query, value):
    """Build the 8 per-core input maps (partition-major fp16 images)."""
    query = np.asarray(query, dtype=np.float32)
    value = np.asarray(value, dtype=np.float32)

    in_maps = []
    for c in range(NCORES):
        b = c % 4
        tiles = _tiles_for_core(c)
        Q = query[b]
        V = value[b]
        # mask threshold is slot-independent: c - r > t*128 + 256 - EXT
        thr = 128 if c < 4 else 0
        r = np.arange(128)[:, None]
        cc = np.arange(256)[None, :]
        mask_img = np.where(cc - r > thr, MASK_VAL, 0.0).astype(np.float16)

        def chunk_img(ch):
            if ch[0] == "mask":
                return mask_img
            if ch[0] == "qt":
                t = tiles[ch[1]]
                qtile = Q[t * 128:(t + 1) * 128, :]  # [128q, 1024d]
                # per-partition layout: [a(d-chunk) major, q-col minor]
                return qtile.T.reshape(8, 128, 128).transpose(1, 0, 2) \
                    .reshape(128, 8 * 128)
            if ch[0] == "vt":
                c0, w = ch[1], ch[2]
                return V[c0:c0 + w, :].T.reshape(8, 128, w) \
                    .transpose(1, 0, 2).reshape(128, 8 * w)
            r0, n = ch[1], ch[2]
            return V[r0:r0 + n * 128, :].reshape(n, 128, D) \
                .transpose(1, 0, 2).reshape(128, n * D)

        m = {}
        for k, wave in enumerate(WAVES):
            m[f"w{k}"] = np.ascontiguousarray(np.hstack(
                [chunk_img(ch) for ch in wave])).astype(np.float16)
        in_maps.append(m)
    return in_maps


def kernel(query, value):
    nc = _get_program()
    in_maps = stage_inputs(query, value)
    res = run_bass_kernel_spmd(nc, in_maps, core_ids=list(range(NCORES)))

    out = np.empty((B, T, D), dtype=np.float32)
    for c in range(NCORES):
        o = np.asarray(res.results[c]["o"], dtype=np.float32)  # [1024, D]
        b = c % 4
        for j, t in enumerate(_tiles_for_core(c)):
            out[b, t * 128:(t + 1) * 128, :] = o[j * 128:(j + 1) * 128, :]
    return out
